# revision 22
# baseline (speedup 1.0000x reference)
"""Trainium2 Bass kernel for nn_EnsembleModel (grouped ensemble dot-product).

Computes out[b, g] = sum_n x[b, g, n] * W[g, n] + b[g] for
x: [16384, 368, 16] f32, W: [368, 16] f32, b: [368] f32.

Strategy: data-parallel over 8 NeuronCores (batch 16384 -> 8 x 2048).
Per core: batch rows on SBUF partitions (contiguous DMA). A custom DVE op
(MAC_SCAN: out = cumsum(x * w) along the free dim, one pass at 1 elem/cyc)
replaces the 2-pass mul+reduce; per-group sums are recovered as strided
differences of the cumulative sum (fp32 error ~1e-5 absolute), then bias.
W/bias are replicated to all 128 partitions on the host (tiny).
"""

import sys

for _p in ("/opt/trn_rl_repo", "/root/.axon_site/_ro/trn_rl_repo"):
    if _p not in sys.path:
        sys.path.append(_p)

import numpy as np

import concourse.bacc as bacc
import concourse.bass as bass
import concourse.mybir as mybir
import concourse.tile as tile
from concourse.bass_utils import run_bass_kernel_spmd

BATCH = 16384
NGROUPS = 368
NMODELS = 16
NCORES = 8
BS = BATCH // NCORES          # 2048 batch rows per core
P = 128                       # SBUF partitions
ROWS_PER_PART = 1             # batch rows packed per partition per tile
TILE_F = NGROUPS * NMODELS * ROWS_PER_PART   # free-dim elems per partition
GOUT = NGROUPS * ROWS_PER_PART               # output elems per partition
NTILES = BS // (P * ROWS_PER_PART)

USE_SCAN = True

_CACHE = {}


def _register_mac_scan():
    """Register the fused multiply+cumsum custom DVE op at runtime."""
    import concourse.dve_ops as dve_ops
    from concourse.dve_ops import DveOp, OPS
    from concourse.dve_spec import AluOp, Spec, Src0, Src1, lower, scan
    from concourse.dve_spec import _has_src1 as has_src1
    from concourse.dve_uop import DveOpSpec

    name = "MAC_SCAN_ANT"
    for op in OPS:
        if op.name == name:
            return op

    def _ref(in0, in1, s0, s1, imm2):
        p = in0.shape[0]
        prod = (np.asarray(in0, np.float32) * np.asarray(in1, np.float32)).reshape(
            p, -1
        )
        return np.cumsum(prod, axis=1, dtype=np.float32).reshape(in0.shape)

    sha = {}
    op = DveOp(
        name,
        Spec(body=scan(AluOp.ADD, Src0 * Src1), reference=_ref),
        subdim=False,
        uops_sha=sha,
    )
    OPS.append(op)
    opcode = dve_ops._CUSTOM_DVE_ROW_BASE + len(OPS) - 1
    dve_ops._SUB_OPCODE_FOR_NAME[name] = opcode
    assert opcode < 0x20
    for ver in ("v3", "v4"):
        uops = lower(op.spec, ver=ver)
        sha[ver] = DveOpSpec(
            name=name, opcode=opcode, uops=uops, rd1_en=has_src1(op.spec)
        ).sha(ver)
    return op


def _build():
    """Build the per-core Bass program (identical on all 8 cores)."""
    mac_scan = _register_mac_scan() if USE_SCAN else None

    nc = bacc.Bacc("TRN2", target_bir_lowering=False, debug=False)
    f32 = mybir.dt.float32

    xs = nc.dram_tensor("x", [BS, NGROUPS * NMODELS], f32, kind="ExternalInput")
    wr = nc.dram_tensor("wrep", [P, TILE_F], f32, kind="ExternalInput")
    br = nc.dram_tensor("brep", [P, GOUT], f32, kind="ExternalInput")
    ys = nc.dram_tensor("y", [BS, NGROUPS], f32, kind="ExternalOutput")

    # tile t, partition p holds batch rows (t*P + p)*ROWS_PER_PART + c
    x_t = xs.ap().rearrange("(t p c) f -> t p (c f)", p=P, c=ROWS_PER_PART)
    y_t = ys.ap().rearrange("(t p c) g -> t p (c g)", p=P, c=ROWS_PER_PART)

    from concourse.tile_rust import add_dep_helper

    NQ = 4                      # startup ramp: quarters for first tile
    FQ = TILE_F // NQ
    GQ = GOUT // NQ
    NE = 8                      # tail ramp: eighths for last tile
    FE = TILE_F // NE
    GE = GOUT // NE

    def scan_diff(xa, wa, oa, gout):
        """cumsum(x*w) in place over xa, then blocked diffs (+first) into oa."""
        nc.vector._custom_dve(mac_scan, out=xa, in0=xa, in1=wa)
        hi = (
            xa.rearrange("p (s n) -> p s n", n=NMODELS)[:, :, NMODELS - 1 : NMODELS]
            .rearrange("p s one -> p (s one)")
        )
        nc.vector.tensor_copy(oa[:, 0:1], hi[:, 0:1])
        nc.vector.tensor_sub(oa[:, 1:gout], hi[:, 1:gout], hi[:, 0 : gout - 1])

    with tile.TileContext(nc) as tc:
        with (
            tc.tile_pool(name="const", bufs=1) as cpool,
            tc.tile_pool(name="x", bufs=6) as xpool,
            tc.tile_pool(name="q", bufs=NQ) as qpool,
            tc.tile_pool(name="o", bufs=6) as opool,
        ):
            # Startup gate: only wq0+xq0 (the first ramp quarter) stream at
            # t=0, so the first scan starts ~5us in. Every later input DMA
            # carries one wait on xq0's completion sem; after that the
            # queues are free to interleave at full depth.
            state = {"gate": None}

            def gated_dma(out_ap, in_ap):
                inst = nc.sync.dma_start(out=out_ap, in_=in_ap)
                if state["gate"] is not None:
                    add_dep_helper(
                        inst.ins, state["gate"].ins, sync=True,
                        reason="startup gate",
                    )
                return inst

            w_tile = cpool.tile([P, TILE_F], f32)
            b_tile = cpool.tile([P, GOUT], f32)

            # ungated: first W quarter (the gate) and the first x quarter;
            # gating on wq0 lets xq0 keep streaming during the ~5us
            # sem-release latency
            sl0 = slice(0, FQ)
            gate_inst = gated_dma(w_tile[:, sl0], wr.ap()[:, sl0])
            t0_q = [qpool.tile([P, FQ], f32, name="xq", tag="xq") for _ in range(NQ)]
            gated_dma(t0_q[0][:], x_t[0][:, sl0])
            state["gate"] = gate_inst

            for q in range(1, NQ):
                sl = slice(q * FQ, (q + 1) * FQ)
                gated_dma(w_tile[:, sl], wr.ap()[:, sl])
                gated_dma(t0_q[q][:], x_t[0][:, sl])
            gated_dma(b_tile[:], br.ap())

            for i in range(NTILES):
                ot = opool.tile([P, GOUT], f32)
                if USE_SCAN and i == 0:
                    for q in range(NQ):
                        sl = slice(q * FQ, (q + 1) * FQ)
                        xq = t0_q[q]
                        scan_diff(
                            xq[:], w_tile[:, sl], ot[:, q * GQ : (q + 1) * GQ], GQ
                        )
                elif USE_SCAN and i == NTILES - 1:
                    # tail ramp: eighths, each with its own bias add + out DMA
                    # so the post-stream chain is ~2us instead of ~5us
                    for e in range(NE):
                        sl = slice(e * FE, (e + 1) * FE)
                        gl = slice(e * GE, (e + 1) * GE)
                        xq = qpool.tile([P, FE], f32, name="xe", tag="xq")
                        gated_dma(xq[:], x_t[i][:, sl])
                        scan_diff(xq[:], w_tile[:, sl], ot[:, gl], GE)
                        nc.vector.tensor_add(ot[:, gl], ot[:, gl], b_tile[:, gl])
                        nc.scalar.dma_start(out=y_t[i][:, gl], in_=ot[:, gl])
                    continue
                elif USE_SCAN:
                    xt = xpool.tile([P, TILE_F], f32)
                    gated_dma(xt[:], x_t[i])
                    scan_diff(xt[:], w_tile[:], ot[:], GOUT)
                else:
                    xt = xpool.tile([P, TILE_F], f32)
                    gated_dma(xt[:], x_t[i])
                    nc.vector.tensor_mul(xt[:], xt[:], w_tile[:])
                    nc.vector.tensor_reduce(
                        ot[:].rearrange("p (c g) -> p c g", c=ROWS_PER_PART),
                        xt[:].rearrange(
                            "p (c g n) -> p (c g) n", c=ROWS_PER_PART, n=NMODELS
                        ),
                        axis=mybir.AxisListType.X,
                        op=mybir.AluOpType.add,
                    )
                nc.vector.tensor_add(ot[:], ot[:], b_tile[:])
                # output DMAs ride the ACT HWDGE ring, off the input queues
                nc.scalar.dma_start(out=y_t[i], in_=ot[:])

    nc.compile()
    return nc


def get_nc():
    if "nc" not in _CACHE:
        _CACHE["nc"] = _build()
    return _CACHE["nc"]


def kernel(x: np.ndarray, W: np.ndarray, b: np.ndarray, trace: bool = False):
    x = np.asarray(x, dtype=np.float32)
    W = np.asarray(W, dtype=np.float32)
    b = np.asarray(b, dtype=np.float32)
    assert x.shape == (BATCH, NGROUPS, NMODELS)

    nc = get_nc()

    wrep = np.ascontiguousarray(
        np.broadcast_to(np.tile(W.reshape(-1).astype(np.float32), ROWS_PER_PART), (P, TILE_F))
    )
    brep = np.ascontiguousarray(
        np.broadcast_to(np.tile(b.astype(np.float32), ROWS_PER_PART), (P, GOUT))
    )

    x2 = x.reshape(BATCH, NGROUPS * NMODELS)
    in_maps = [
        {"x": x2[c * BS : (c + 1) * BS], "wrep": wrep, "brep": brep}
        for c in range(NCORES)
    ]

    res = run_bass_kernel_spmd(
        nc, in_maps, core_ids=list(range(NCORES)), trace=trace
    )
    out = np.concatenate([res.results[c]["y"] for c in range(NCORES)], axis=0)
    if trace:
        kernel.last_exec_time_ns = res.exec_time_ns
        kernel.last_results = res
    return out


kernel.last_exec_time_ns = None
kernel.last_results = None


# revision 23
# speedup vs baseline: 1.0186x; 1.0186x over previous
"""Trainium2 Bass kernel for nn_EnsembleModel (grouped ensemble dot-product).

Computes out[b, g] = sum_n x[b, g, n] * W[g, n] + b[g] for
x: [16384, 368, 16] f32, W: [368, 16] f32, b: [368] f32.

Strategy: data-parallel over 8 NeuronCores (batch 16384 -> 8 x 2048).
Per core: batch rows on SBUF partitions (contiguous DMA). A custom DVE op
(MAC_SCAN: out = cumsum(x * w) along the free dim, one pass at 1 elem/cyc)
replaces the 2-pass mul+reduce; per-group sums are recovered as strided
differences of the cumulative sum (fp32 error ~1e-5 absolute), then bias.
W/bias are replicated to all 128 partitions on the host (tiny).
"""

import sys

for _p in ("/opt/trn_rl_repo", "/root/.axon_site/_ro/trn_rl_repo"):
    if _p not in sys.path:
        sys.path.append(_p)

import numpy as np

import concourse.bacc as bacc
import concourse.bass as bass
import concourse.mybir as mybir
import concourse.tile as tile
from concourse.bass_utils import run_bass_kernel_spmd

BATCH = 16384
NGROUPS = 368
NMODELS = 16
NCORES = 8
BS = BATCH // NCORES          # 2048 batch rows per core
P = 128                       # SBUF partitions
ROWS_PER_PART = 1             # batch rows packed per partition per tile
TILE_F = NGROUPS * NMODELS * ROWS_PER_PART   # free-dim elems per partition
GOUT = NGROUPS * ROWS_PER_PART               # output elems per partition
NTILES = BS // (P * ROWS_PER_PART)

USE_SCAN = True

_CACHE = {}


def _register_mac_scan():
    """Register the fused multiply+cumsum custom DVE op at runtime."""
    import concourse.dve_ops as dve_ops
    from concourse.dve_ops import DveOp, OPS
    from concourse.dve_spec import AluOp, Spec, Src0, Src1, lower, scan
    from concourse.dve_spec import _has_src1 as has_src1
    from concourse.dve_uop import DveOpSpec

    name = "MAC_SCAN_ANT"
    for op in OPS:
        if op.name == name:
            return op

    def _ref(in0, in1, s0, s1, imm2):
        p = in0.shape[0]
        prod = (np.asarray(in0, np.float32) * np.asarray(in1, np.float32)).reshape(
            p, -1
        )
        return np.cumsum(prod, axis=1, dtype=np.float32).reshape(in0.shape)

    sha = {}
    op = DveOp(
        name,
        Spec(body=scan(AluOp.ADD, Src0 * Src1), reference=_ref),
        subdim=False,
        uops_sha=sha,
    )
    OPS.append(op)
    opcode = dve_ops._CUSTOM_DVE_ROW_BASE + len(OPS) - 1
    dve_ops._SUB_OPCODE_FOR_NAME[name] = opcode
    assert opcode < 0x20
    for ver in ("v3", "v4"):
        uops = lower(op.spec, ver=ver)
        sha[ver] = DveOpSpec(
            name=name, opcode=opcode, uops=uops, rd1_en=has_src1(op.spec)
        ).sha(ver)
    return op


def _build():
    """Build the per-core Bass program (identical on all 8 cores)."""
    mac_scan = _register_mac_scan() if USE_SCAN else None

    nc = bacc.Bacc("TRN2", target_bir_lowering=False, debug=False)
    f32 = mybir.dt.float32

    xs = nc.dram_tensor("x", [BS, NGROUPS * NMODELS], f32, kind="ExternalInput")
    wr = nc.dram_tensor("wrep", [P, TILE_F], f32, kind="ExternalInput")
    br = nc.dram_tensor("brep", [P, GOUT], f32, kind="ExternalInput")
    ys = nc.dram_tensor("y", [BS, NGROUPS], f32, kind="ExternalOutput")

    # tile t, partition p holds batch rows (t*P + p)*ROWS_PER_PART + c
    x_t = xs.ap().rearrange("(t p c) f -> t p (c f)", p=P, c=ROWS_PER_PART)
    y_t = ys.ap().rearrange("(t p c) g -> t p (c g)", p=P, c=ROWS_PER_PART)

    from concourse.tile_rust import add_dep_helper

    NQ = 4                      # startup/tail ramp: quarters for first/last tile
    FQ = TILE_F // NQ
    GQ = GOUT // NQ
    RAMP_TILES = (0, NTILES - 1)

    def scan_diff(xa, wa, oa, gout):
        """cumsum(x*w) in place over xa, then blocked diffs (+first) into oa."""
        nc.vector._custom_dve(mac_scan, out=xa, in0=xa, in1=wa)
        hi = (
            xa.rearrange("p (s n) -> p s n", n=NMODELS)[:, :, NMODELS - 1 : NMODELS]
            .rearrange("p s one -> p (s one)")
        )
        nc.vector.tensor_copy(oa[:, 0:1], hi[:, 0:1])
        nc.vector.tensor_sub(oa[:, 1:gout], hi[:, 1:gout], hi[:, 0 : gout - 1])

    with tile.TileContext(nc) as tc:
        with (
            tc.tile_pool(name="const", bufs=1) as cpool,
            tc.tile_pool(name="x", bufs=6) as xpool,
            tc.tile_pool(name="q", bufs=NQ) as qpool,
            tc.tile_pool(name="o", bufs=6) as opool,
        ):
            # Startup gate: only wq0+xq0 (the first ramp quarter) stream at
            # t=0, so the first scan starts ~5us in. Every later input DMA
            # carries one wait on xq0's completion sem; after that the
            # queues are free to interleave at full depth.
            state = {"gate": None}

            def gated_dma(out_ap, in_ap):
                inst = nc.sync.dma_start(out=out_ap, in_=in_ap)
                if state["gate"] is not None:
                    add_dep_helper(
                        inst.ins, state["gate"].ins, sync=True,
                        reason="startup gate",
                    )
                return inst

            w_tile = cpool.tile([P, TILE_F], f32)
            b_tile = cpool.tile([P, GOUT], f32)

            # ungated: first W quarter (the gate) and the first x quarter;
            # gating on wq0 lets xq0 keep streaming during the ~5us
            # sem-release latency
            sl0 = slice(0, FQ)
            gate_inst = gated_dma(w_tile[:, sl0], wr.ap()[:, sl0])
            t0_q = [qpool.tile([P, FQ], f32, name="xq", tag="xq") for _ in range(NQ)]
            gated_dma(t0_q[0][:], x_t[0][:, sl0])
            state["gate"] = gate_inst

            for q in range(1, NQ):
                sl = slice(q * FQ, (q + 1) * FQ)
                gated_dma(w_tile[:, sl], wr.ap()[:, sl])
                gated_dma(t0_q[q][:], x_t[0][:, sl])
            gated_dma(b_tile[:], br.ap())

            for i in range(NTILES):
                ot = opool.tile([P, GOUT], f32)
                if USE_SCAN and i in RAMP_TILES:
                    for q in range(NQ):
                        sl = slice(q * FQ, (q + 1) * FQ)
                        if i == 0:
                            xq = t0_q[q]
                        else:
                            xq = qpool.tile([P, FQ], f32, name="xq", tag="xq")
                            gated_dma(xq[:], x_t[i][:, sl])
                        scan_diff(
                            xq[:], w_tile[:, sl], ot[:, q * GQ : (q + 1) * GQ], GQ
                        )
                elif USE_SCAN:
                    xt = xpool.tile([P, TILE_F], f32)
                    gated_dma(xt[:], x_t[i])
                    scan_diff(xt[:], w_tile[:], ot[:], GOUT)
                else:
                    xt = xpool.tile([P, TILE_F], f32)
                    gated_dma(xt[:], x_t[i])
                    nc.vector.tensor_mul(xt[:], xt[:], w_tile[:])
                    nc.vector.tensor_reduce(
                        ot[:].rearrange("p (c g) -> p c g", c=ROWS_PER_PART),
                        xt[:].rearrange(
                            "p (c g n) -> p (c g) n", c=ROWS_PER_PART, n=NMODELS
                        ),
                        axis=mybir.AxisListType.X,
                        op=mybir.AluOpType.add,
                    )
                nc.vector.tensor_add(ot[:], ot[:], b_tile[:])
                # output DMAs ride the ACT HWDGE ring, off the input queues
                nc.scalar.dma_start(out=y_t[i], in_=ot[:])

    nc.compile()
    return nc


def get_nc():
    if "nc" not in _CACHE:
        _CACHE["nc"] = _build()
    return _CACHE["nc"]


def kernel(x: np.ndarray, W: np.ndarray, b: np.ndarray, trace: bool = False):
    x = np.asarray(x, dtype=np.float32)
    W = np.asarray(W, dtype=np.float32)
    b = np.asarray(b, dtype=np.float32)
    assert x.shape == (BATCH, NGROUPS, NMODELS)

    nc = get_nc()

    wrep = np.ascontiguousarray(
        np.broadcast_to(np.tile(W.reshape(-1).astype(np.float32), ROWS_PER_PART), (P, TILE_F))
    )
    brep = np.ascontiguousarray(
        np.broadcast_to(np.tile(b.astype(np.float32), ROWS_PER_PART), (P, GOUT))
    )

    x2 = x.reshape(BATCH, NGROUPS * NMODELS)
    in_maps = [
        {"x": x2[c * BS : (c + 1) * BS], "wrep": wrep, "brep": brep}
        for c in range(NCORES)
    ]

    res = run_bass_kernel_spmd(
        nc, in_maps, core_ids=list(range(NCORES)), trace=trace
    )
    out = np.concatenate([res.results[c]["y"] for c in range(NCORES)], axis=0)
    if trace:
        kernel.last_exec_time_ns = res.exec_time_ns
        kernel.last_results = res
    return out


kernel.last_exec_time_ns = None
kernel.last_results = None


# revision 24
# speedup vs baseline: 1.0315x; 1.0127x over previous
"""Trainium2 Bass kernel for nn_EnsembleModel (grouped ensemble dot-product).

Computes out[b, g] = sum_n x[b, g, n] * W[g, n] + b[g] for
x: [16384, 368, 16] f32, W: [368, 16] f32, b: [368] f32.

Strategy: data-parallel over 8 NeuronCores (batch 16384 -> 8 x 2048).
Per core: batch rows on SBUF partitions (contiguous DMA). A custom DVE op
(MAC_SCAN: out = cumsum(x * w) along the free dim, one pass at 1 elem/cyc)
replaces the 2-pass mul+reduce; per-group sums are recovered as strided
differences of the cumulative sum (fp32 error ~1e-5 absolute), then bias.
W/bias are replicated to all 128 partitions on the host (tiny).
"""

import sys

for _p in ("/opt/trn_rl_repo", "/root/.axon_site/_ro/trn_rl_repo"):
    if _p not in sys.path:
        sys.path.append(_p)

import numpy as np

import concourse.bacc as bacc
import concourse.bass as bass
import concourse.mybir as mybir
import concourse.tile as tile
from concourse.bass_utils import run_bass_kernel_spmd

BATCH = 16384
NGROUPS = 368
NMODELS = 16
NCORES = 8
BS = BATCH // NCORES          # 2048 batch rows per core
P = 128                       # SBUF partitions
ROWS_PER_PART = 1             # batch rows packed per partition per tile
TILE_F = NGROUPS * NMODELS * ROWS_PER_PART   # free-dim elems per partition
GOUT = NGROUPS * ROWS_PER_PART               # output elems per partition
NTILES = BS // (P * ROWS_PER_PART)

USE_SCAN = True

_CACHE = {}


def _register_mac_scan():
    """Register the fused multiply+cumsum custom DVE op at runtime."""
    import concourse.dve_ops as dve_ops
    from concourse.dve_ops import DveOp, OPS
    from concourse.dve_spec import AluOp, Spec, Src0, Src1, lower, scan
    from concourse.dve_spec import _has_src1 as has_src1
    from concourse.dve_uop import DveOpSpec

    name = "MAC_SCAN_ANT"
    for op in OPS:
        if op.name == name:
            return op

    def _ref(in0, in1, s0, s1, imm2):
        p = in0.shape[0]
        prod = (np.asarray(in0, np.float32) * np.asarray(in1, np.float32)).reshape(
            p, -1
        )
        return np.cumsum(prod, axis=1, dtype=np.float32).reshape(in0.shape)

    sha = {}
    op = DveOp(
        name,
        Spec(body=scan(AluOp.ADD, Src0 * Src1), reference=_ref),
        subdim=False,
        uops_sha=sha,
    )
    OPS.append(op)
    opcode = dve_ops._CUSTOM_DVE_ROW_BASE + len(OPS) - 1
    dve_ops._SUB_OPCODE_FOR_NAME[name] = opcode
    assert opcode < 0x20
    for ver in ("v3", "v4"):
        uops = lower(op.spec, ver=ver)
        sha[ver] = DveOpSpec(
            name=name, opcode=opcode, uops=uops, rd1_en=has_src1(op.spec)
        ).sha(ver)
    return op


def _build():
    """Build the per-core Bass program (identical on all 8 cores)."""
    mac_scan = _register_mac_scan() if USE_SCAN else None

    nc = bacc.Bacc("TRN2", target_bir_lowering=False, debug=False)
    f32 = mybir.dt.float32

    xs = nc.dram_tensor("x", [BS, NGROUPS * NMODELS], f32, kind="ExternalInput")
    wr = nc.dram_tensor("wrep", [P, TILE_F], f32, kind="ExternalInput")
    br = nc.dram_tensor("brep", [P, GOUT], f32, kind="ExternalInput")
    ys = nc.dram_tensor("y", [BS, NGROUPS], f32, kind="ExternalOutput")

    # tile t, partition p holds batch rows (t*P + p)*ROWS_PER_PART + c
    x_t = xs.ap().rearrange("(t p c) f -> t p (c f)", p=P, c=ROWS_PER_PART)
    y_t = ys.ap().rearrange("(t p c) g -> t p (c g)", p=P, c=ROWS_PER_PART)

    from concourse.tile_rust import add_dep_helper

    NQ = 4                      # startup/tail ramp: quarters for first/last tile
    FQ = TILE_F // NQ
    GQ = GOUT // NQ
    RAMP_TILES = (0, NTILES - 1)

    def scan_diff(xa, wa, oa, gout):
        """cumsum(x*w) in place over xa, then blocked diffs (+first) into oa."""
        nc.vector._custom_dve(mac_scan, out=xa, in0=xa, in1=wa)
        hi = (
            xa.rearrange("p (s n) -> p s n", n=NMODELS)[:, :, NMODELS - 1 : NMODELS]
            .rearrange("p s one -> p (s one)")
        )
        nc.vector.tensor_copy(oa[:, 0:1], hi[:, 0:1])
        nc.vector.tensor_sub(oa[:, 1:gout], hi[:, 1:gout], hi[:, 0 : gout - 1])

    with tile.TileContext(nc) as tc:
        with (
            tc.tile_pool(name="const", bufs=1) as cpool,
            tc.tile_pool(name="x", bufs=6) as xpool,
            tc.tile_pool(name="q", bufs=NQ) as qpool,
            tc.tile_pool(name="o", bufs=6) as opool,
        ):
            # Startup gate: only wq0+xq0 (the first ramp quarter) stream at
            # t=0, so the first scan starts ~5us in. Every later input DMA
            # carries one wait on xq0's completion sem; after that the
            # queues are free to interleave at full depth.
            state = {"gate": None}

            def gated_dma(out_ap, in_ap):
                inst = nc.sync.dma_start(out=out_ap, in_=in_ap)
                if state["gate"] is not None:
                    add_dep_helper(
                        inst.ins, state["gate"].ins, sync=True,
                        reason="startup gate",
                    )
                return inst

            w_tile = cpool.tile([P, TILE_F], f32)
            b_tile = cpool.tile([P, GOUT], f32)

            # ungated: first W quarter (the gate) and the first x quarter;
            # gating on wq0 lets xq0 keep streaming during the ~5us
            # sem-release latency
            sl0 = slice(0, FQ)
            gate_inst = gated_dma(w_tile[:, sl0], wr.ap()[:, sl0])
            t0_q = [qpool.tile([P, FQ], f32, name="xq", tag="xq") for _ in range(NQ)]
            gated_dma(t0_q[0][:], x_t[0][:, sl0])
            # tile 1's full load also ungated: it fills the ~4us the engines
            # would otherwise idle while the gate sem releases
            xt1 = xpool.tile([P, TILE_F], f32, name="xt", tag="xt")
            gated_dma(xt1[:], x_t[1])
            state["gate"] = gate_inst

            for q in range(1, NQ):
                sl = slice(q * FQ, (q + 1) * FQ)
                gated_dma(w_tile[:, sl], wr.ap()[:, sl])
                gated_dma(t0_q[q][:], x_t[0][:, sl])
            gated_dma(b_tile[:], br.ap())

            for i in range(NTILES):
                ot = opool.tile([P, GOUT], f32)
                if USE_SCAN and i in RAMP_TILES:
                    for q in range(NQ):
                        sl = slice(q * FQ, (q + 1) * FQ)
                        if i == 0:
                            xq = t0_q[q]
                        else:
                            xq = qpool.tile([P, FQ], f32, name="xq", tag="xq")
                            gated_dma(xq[:], x_t[i][:, sl])
                        scan_diff(
                            xq[:], w_tile[:, sl], ot[:, q * GQ : (q + 1) * GQ], GQ
                        )
                elif USE_SCAN:
                    if i == 1:
                        xt = xt1
                    else:
                        xt = xpool.tile([P, TILE_F], f32, name="xt", tag="xt")
                        gated_dma(xt[:], x_t[i])
                    scan_diff(xt[:], w_tile[:], ot[:], GOUT)
                else:
                    xt = xpool.tile([P, TILE_F], f32)
                    gated_dma(xt[:], x_t[i])
                    nc.vector.tensor_mul(xt[:], xt[:], w_tile[:])
                    nc.vector.tensor_reduce(
                        ot[:].rearrange("p (c g) -> p c g", c=ROWS_PER_PART),
                        xt[:].rearrange(
                            "p (c g n) -> p (c g) n", c=ROWS_PER_PART, n=NMODELS
                        ),
                        axis=mybir.AxisListType.X,
                        op=mybir.AluOpType.add,
                    )
                nc.vector.tensor_add(ot[:], ot[:], b_tile[:])
                # output DMAs ride the ACT HWDGE ring, off the input queues
                nc.scalar.dma_start(out=y_t[i], in_=ot[:])

    nc.compile()
    return nc


def get_nc():
    if "nc" not in _CACHE:
        _CACHE["nc"] = _build()
    return _CACHE["nc"]


def kernel(x: np.ndarray, W: np.ndarray, b: np.ndarray, trace: bool = False):
    x = np.asarray(x, dtype=np.float32)
    W = np.asarray(W, dtype=np.float32)
    b = np.asarray(b, dtype=np.float32)
    assert x.shape == (BATCH, NGROUPS, NMODELS)

    nc = get_nc()

    wrep = np.ascontiguousarray(
        np.broadcast_to(np.tile(W.reshape(-1).astype(np.float32), ROWS_PER_PART), (P, TILE_F))
    )
    brep = np.ascontiguousarray(
        np.broadcast_to(np.tile(b.astype(np.float32), ROWS_PER_PART), (P, GOUT))
    )

    x2 = x.reshape(BATCH, NGROUPS * NMODELS)
    in_maps = [
        {"x": x2[c * BS : (c + 1) * BS], "wrep": wrep, "brep": brep}
        for c in range(NCORES)
    ]

    res = run_bass_kernel_spmd(
        nc, in_maps, core_ids=list(range(NCORES)), trace=trace
    )
    out = np.concatenate([res.results[c]["y"] for c in range(NCORES)], axis=0)
    if trace:
        kernel.last_exec_time_ns = res.exec_time_ns
        kernel.last_results = res
    return out


kernel.last_exec_time_ns = None
kernel.last_results = None


# revision 25
# speedup vs baseline: 1.0315x; 1.0000x over previous
"""Trainium2 Bass kernel for nn_EnsembleModel (grouped ensemble dot-product).

Computes out[b, g] = sum_n x[b, g, n] * W[g, n] + b[g] for
x: [16384, 368, 16] f32, W: [368, 16] f32, b: [368] f32.

Strategy: data-parallel over 8 NeuronCores (batch 16384 -> 8 x 2048).
Per core: batch rows on SBUF partitions (contiguous DMA). A custom DVE op
(MAC_SCAN: out = cumsum(x * w) along the free dim, one pass at 1 elem/cyc)
replaces the 2-pass mul+reduce; per-group sums are recovered as strided
differences of the cumulative sum (fp32 error ~1e-5 absolute), then bias.
W/bias are replicated to all 128 partitions on the host (tiny).
"""

import sys

for _p in ("/opt/trn_rl_repo", "/root/.axon_site/_ro/trn_rl_repo"):
    if _p not in sys.path:
        sys.path.append(_p)

import numpy as np

import concourse.bacc as bacc
import concourse.bass as bass
import concourse.mybir as mybir
import concourse.tile as tile
from concourse.bass_utils import run_bass_kernel_spmd

BATCH = 16384
NGROUPS = 368
NMODELS = 16
NCORES = 8
BS = BATCH // NCORES          # 2048 batch rows per core
P = 128                       # SBUF partitions
ROWS_PER_PART = 1             # batch rows packed per partition per tile
TILE_F = NGROUPS * NMODELS * ROWS_PER_PART   # free-dim elems per partition
TILE_FP = TILE_F + NMODELS                   # + 16-elem zero-block prefix
GOUT = NGROUPS * ROWS_PER_PART               # output elems per partition
NTILES = BS // (P * ROWS_PER_PART)

USE_SCAN = True

_CACHE = {}


def _register_mac_scan():
    """Register the fused multiply+cumsum custom DVE op at runtime."""
    import concourse.dve_ops as dve_ops
    from concourse.dve_ops import DveOp, OPS
    from concourse.dve_spec import AluOp, Spec, Src0, Src1, lower, scan
    from concourse.dve_spec import _has_src1 as has_src1
    from concourse.dve_uop import DveOpSpec

    name = "MAC_SCAN_ANT"
    for op in OPS:
        if op.name == name:
            return op

    def _ref(in0, in1, s0, s1, imm2):
        p = in0.shape[0]
        prod = (np.asarray(in0, np.float32) * np.asarray(in1, np.float32)).reshape(
            p, -1
        )
        return np.cumsum(prod, axis=1, dtype=np.float32).reshape(in0.shape)

    sha = {}
    op = DveOp(
        name,
        Spec(body=scan(AluOp.ADD, Src0 * Src1), reference=_ref),
        subdim=False,
        uops_sha=sha,
    )
    OPS.append(op)
    opcode = dve_ops._CUSTOM_DVE_ROW_BASE + len(OPS) - 1
    dve_ops._SUB_OPCODE_FOR_NAME[name] = opcode
    assert opcode < 0x20
    for ver in ("v3", "v4"):
        uops = lower(op.spec, ver=ver)
        sha[ver] = DveOpSpec(
            name=name, opcode=opcode, uops=uops, rd1_en=has_src1(op.spec)
        ).sha(ver)
    return op


def _build():
    """Build the per-core Bass program (identical on all 8 cores)."""
    mac_scan = _register_mac_scan() if USE_SCAN else None

    nc = bacc.Bacc("TRN2", target_bir_lowering=False, debug=False)
    f32 = mybir.dt.float32

    xs = nc.dram_tensor("x", [BS, NGROUPS * NMODELS], f32, kind="ExternalInput")
    wr = nc.dram_tensor("wrep", [P, TILE_FP], f32, kind="ExternalInput")
    br = nc.dram_tensor("brep", [P, GOUT], f32, kind="ExternalInput")
    ys = nc.dram_tensor("y", [BS, NGROUPS], f32, kind="ExternalOutput")

    # tile t, partition p holds batch rows (t*P + p)*ROWS_PER_PART + c
    x_t = xs.ap().rearrange("(t p c) f -> t p (c f)", p=P, c=ROWS_PER_PART)
    y_t = ys.ap().rearrange("(t p c) g -> t p (c g)", p=P, c=ROWS_PER_PART)

    from concourse.tile_rust import add_dep_helper

    NQ = 4                      # startup/tail ramp: quarters for first/last tile
    FQ = TILE_F // NQ
    GQ = GOUT // NQ
    RAMP_TILES = (0, NTILES - 1)

    def scan_diff(xa, wa, oa, gout):
        """cumsum(x*w) in place over xa, then blocked diffs (+first) into oa."""
        nc.vector._custom_dve(mac_scan, out=xa, in0=xa, in1=wa)
        hi = (
            xa.rearrange("p (s n) -> p s n", n=NMODELS)[:, :, NMODELS - 1 : NMODELS]
            .rearrange("p s one -> p (s one)")
        )
        nc.vector.tensor_copy(oa[:, 0:1], hi[:, 0:1])
        nc.vector.tensor_sub(oa[:, 1:gout], hi[:, 1:gout], hi[:, 0 : gout - 1])

    with tile.TileContext(nc) as tc:
        with (
            tc.tile_pool(name="const", bufs=1) as cpool,
            tc.tile_pool(name="x", bufs=6) as xpool,
            tc.tile_pool(name="q", bufs=NQ) as qpool,
            tc.tile_pool(name="o", bufs=6) as opool,
        ):
            # Startup gate: only wq0+xq0 (the first ramp quarter) stream at
            # t=0, so the first scan starts ~5us in. Every later input DMA
            # carries one wait on xq0's completion sem; after that the
            # queues are free to interleave at full depth.
            state = {"gate": None}

            def gated_dma(out_ap, in_ap):
                inst = nc.sync.dma_start(out=out_ap, in_=in_ap)
                if state["gate"] is not None:
                    add_dep_helper(
                        inst.ins, state["gate"].ins, sync=True,
                        reason="startup gate",
                    )
                return inst

            w_tile = cpool.tile([P, TILE_FP], f32)
            b_tile = cpool.tile([P, GOUT], f32)
            # W chunk q covers the zero prefix + ramp-quarter q's weights
            WCH = [(0, NMODELS + FQ)] + [
                (NMODELS + q * FQ, NMODELS + (q + 1) * FQ) for q in range(1, NQ)
            ]

            def x_full_src(i):
                # full-tile source: start 16 elems early so the scan's first
                # block (x * zero-weights) lands a leading 0 in the cumsum
                return bass.AP(
                    xs.ap().tensor,
                    i * P * TILE_F - NMODELS,
                    [[TILE_F, P], [1, TILE_FP]],
                )

            # ungated: first W quarter (the gate) and the first x quarter;
            # gating on wq0 lets xq0 keep streaming during the ~5us
            # sem-release latency
            c0 = slice(*WCH[0])
            gate_inst = gated_dma(w_tile[:, c0], wr.ap()[:, c0])
            t0_q = [qpool.tile([P, FQ], f32, name="xq", tag="xq") for _ in range(NQ)]
            gated_dma(t0_q[0][:], x_t[0][:, 0:FQ])
            # tile 1's full load also ungated: it fills the ~4us the engines
            # would otherwise idle while the gate sem releases
            xt1 = xpool.tile([P, TILE_FP], f32, name="xt", tag="xt")
            gated_dma(xt1[:], x_full_src(1))
            state["gate"] = gate_inst

            for q in range(1, NQ):
                cq = slice(*WCH[q])
                gated_dma(w_tile[:, cq], wr.ap()[:, cq])
                gated_dma(t0_q[q][:], x_t[0][:, q * FQ : (q + 1) * FQ])
            gated_dma(b_tile[:], br.ap())

            for i in range(NTILES):
                ot = opool.tile([P, GOUT], f32)
                if USE_SCAN and i in RAMP_TILES:
                    for q in range(NQ):
                        sl = slice(q * FQ, (q + 1) * FQ)
                        if i == 0:
                            xq = t0_q[q]
                        else:
                            xq = qpool.tile([P, FQ], f32, name="xq", tag="xq")
                            gated_dma(xq[:], x_t[i][:, sl])
                        wsl = slice(NMODELS + q * FQ, NMODELS + (q + 1) * FQ)
                        scan_diff(
                            xq[:], w_tile[:, wsl], ot[:, q * GQ : (q + 1) * GQ], GQ
                        )
                elif USE_SCAN:
                    if i == 1:
                        xt = xt1
                    else:
                        xt = xpool.tile([P, TILE_FP], f32, name="xt", tag="xt")
                        gated_dma(xt[:], x_full_src(i))
                    # zero-prefixed cumsum: one SUB yields all 368 diffs
                    nc.vector._custom_dve(mac_scan, out=xt[:], in0=xt[:], in1=w_tile[:])
                    hi = (
                        xt[:]
                        .rearrange("p (s n) -> p s n", n=NMODELS)[:, :, NMODELS - 1 : NMODELS]
                        .rearrange("p s one -> p (s one)")
                    )
                    nc.vector.tensor_sub(ot[:], hi[:, 1 : GOUT + 1], hi[:, 0:GOUT])
                else:
                    xt = xpool.tile([P, TILE_F], f32)
                    gated_dma(xt[:], x_t[i])
                    nc.vector.tensor_mul(xt[:], xt[:], w_tile[:])
                    nc.vector.tensor_reduce(
                        ot[:].rearrange("p (c g) -> p c g", c=ROWS_PER_PART),
                        xt[:].rearrange(
                            "p (c g n) -> p (c g) n", c=ROWS_PER_PART, n=NMODELS
                        ),
                        axis=mybir.AxisListType.X,
                        op=mybir.AluOpType.add,
                    )
                nc.vector.tensor_add(ot[:], ot[:], b_tile[:])
                # output DMAs ride the ACT HWDGE ring, off the input queues
                nc.scalar.dma_start(out=y_t[i], in_=ot[:])

    nc.compile()
    return nc


def get_nc():
    if "nc" not in _CACHE:
        _CACHE["nc"] = _build()
    return _CACHE["nc"]


def kernel(x: np.ndarray, W: np.ndarray, b: np.ndarray, trace: bool = False):
    x = np.asarray(x, dtype=np.float32)
    W = np.asarray(W, dtype=np.float32)
    b = np.asarray(b, dtype=np.float32)
    assert x.shape == (BATCH, NGROUPS, NMODELS)

    nc = get_nc()

    wflat = np.concatenate(
        [np.zeros(NMODELS, np.float32), np.tile(W.reshape(-1).astype(np.float32), ROWS_PER_PART)]
    )
    wrep = np.ascontiguousarray(np.broadcast_to(wflat, (P, TILE_FP)))
    brep = np.ascontiguousarray(
        np.broadcast_to(np.tile(b.astype(np.float32), ROWS_PER_PART), (P, GOUT))
    )

    x2 = x.reshape(BATCH, NGROUPS * NMODELS)
    in_maps = [
        {"x": x2[c * BS : (c + 1) * BS], "wrep": wrep, "brep": brep}
        for c in range(NCORES)
    ]

    res = run_bass_kernel_spmd(
        nc, in_maps, core_ids=list(range(NCORES)), trace=trace
    )
    out = np.concatenate([res.results[c]["y"] for c in range(NCORES)], axis=0)
    if trace:
        kernel.last_exec_time_ns = res.exec_time_ns
        kernel.last_results = res
    return out


kernel.last_exec_time_ns = None
kernel.last_results = None
